# revision 2
# baseline (speedup 1.0000x reference)
"""BoundaryLoss TRN2 kernel — 8-core SPMD Bass/Tile implementation.

Strategy (data-parallel over batch B=8, one batch per NeuronCore):
  * Only every 4th pixel of preds/fsss_gts matters (nearest 512->128 downsample
    = stride-4 selection), so the host shards preds[:, :, ::4, ::4] etc.
  * The scatter into the circular memory banks only matters through column
    sums: sum(pos_mem) = sum(pos_memory[n_pos:]) + sum(selected E rows),
    n_pos = min(#selected, 333).  No scatter is materialised.
  * first-k selection over the global batch-major flattening = per-core
    cumsum + exclusive prefix of per-core counts (one tiny AllGather).
  * Per core: weighted sums S_x = sum_n sel_x[n] * E[n, :] computed as 128
    PE matmuls over transposed embedding tiles with per-position weights
    sel_x * 1/max(||emb_n||, eps); one AllReduce of the [3,128] partials;
    the final scalar is computed redundantly on every core.

Self-contained: hardcodes all shapes for the nn_BoundaryLoss problem
(B=8, C=21, H=W=512, D=128, h=w=128, MEM=1000).
"""

import ml_dtypes
import numpy as np

import concourse.bass as bass
import concourse.bacc as bacc
import concourse.mybir as mybir
import concourse.tile as tile
from concourse.bass_utils import run_bass_kernel_spmd
from concourse.tile_rust import add_dep_helper

F32 = mybir.dt.float32
BF16 = mybir.dt.bfloat16
I32 = mybir.dt.int32
U32 = mybir.dt.uint32
Alu = mybir.AluOpType
AX = mybir.AxisListType

N_CORES = 8
CORE_IDS = list(range(N_CORES))
P = 128          # partitions / feature spatial dim
NCH = 21         # pred classes
D = 128          # embedding dim
MEM = 1000
K_ANC, K_POS, K_NEG = MEM // 10, MEM // 3, MEM // 3
MARGIN = 0.2
EPS = 1e-12
NCHUNK = 8       # embedding chunks per core (16 tiles of 128 positions each)
TPC = 16         # tiles per chunk


def _build():
    nc = bacc.Bacc("TRN2", target_bir_lowering=False, debug=False,
                   num_devices=N_CORES)

    # ---- per-core sharded inputs ----
    preds_in = nc.declare_dram_parameter("p_preds", [P, NCH * P], F32, isOutput=False)
    fsss_in = nc.declare_dram_parameter("p_fsss", [P, P], I32, isOutput=False)
    emb_in = nc.declare_dram_parameter("p_emb", [NCHUNK, P, TPC * D], BF16, isOutput=False)
    pmem_in = nc.declare_dram_parameter("p_pmem", [P, 8 * D], BF16, isOutput=False)
    nmem_in = nc.declare_dram_parameter("p_nmem", [P, 8 * D], BF16, isOutput=False)
    # ---- replicated constants (one packed tensor) ----
    cst_in = nc.declare_dram_parameter("c_all", [P, 400], F32, isOutput=False)

    out_t = nc.declare_dram_parameter("out", [1, 1], F32, isOutput=True)

    with tile.TileContext(nc) as tc:
        with (
            tc.tile_pool(name="sb", bufs=1) as sb,
            tc.tile_pool(name="stream", bufs=8) as stream,
            tc.tile_pool(name="sq", bufs=2) as sqp,
            tc.tile_pool(name="ps", bufs=1, space="PSUM") as ps,
            tc.tile_pool(name="pst", bufs=2, space="PSUM") as pst,
            tc.tile_pool(name="dr", bufs=1, space="DRAM") as dr,
        ):
            # Dummy collective: pays the one-time collective bootstrap /
            # cross-core sync cost concurrently with the mask+norm phase,
            # so the real AllGather later is fast.
            warm_in_d = dr.tile([1, 32], F32)
            warm_out_d = dr.tile([8, 32], F32, addr_space="Shared")
            warm_cc = nc.gpsimd.collective_compute(
                "AllGather", Alu.bypass, replica_groups=[CORE_IDS],
                ins=[warm_in_d[:]], outs=[warm_out_d[:]])

            # preds/fsss first in the DMA queue (mask phase is the critical path)
            preds_sb = sb.tile([P, NCH * P], F32)
            nc.sync.dma_start(out=preds_sb[:], in_=preds_in[:])
            fsss_sb = sb.tile([P, P], I32)
            nc.sync.dma_start(out=fsss_sb[:], in_=fsss_in[:])

            # ============== constants (single DMA) ==============
            cst = sb.tile([P, 400], F32)
            nc.sync.dma_start(out=cst[:], in_=cst_in[:])
            ident = cst[:, 0:128]
            lstrict = cst[:, 128:256]
            iotam = cst[:, 256:264]
            onescol = cst[:, 264:265]
            ones8 = cst[0:8, 264:265]
            onesrow = cst[0:1, 265:393]
            ones8r = cst[0:1, 265:273]
            iota8 = cst[0:8, 393:394]
            k3row = cst[0:1, 394:397]

            # ============== mask phase (grids are [h, w]) ==============
            preds_re = preds_sb[:].rearrange("p (c w) -> p w c", c=NCH)
            mx = sb.tile([P, P], F32)
            nc.vector.reduce_max(out=mx[:], in_=preds_re[:, :, 1:NCH], axis=AX.X)
            pm = sb.tile([P, P], F32)  # pred_mask
            nc.vector.tensor_tensor(out=pm[:], in0=mx[:], in1=preds_re[:, :, 0],
                                    op=Alu.is_gt)

            fsf = sb.tile([P, P], F32)
            nc.vector.tensor_copy(fsf[:], fsss_sb[:])
            e0 = sb.tile([P, P], F32)
            nc.vector.tensor_scalar(out=e0[:], in0=fsf[:], scalar1=0.0,
                                    scalar2=None, op0=Alu.is_equal)
            e255 = sb.tile([P, P], F32)
            nc.vector.tensor_scalar(out=e255[:], in0=fsf[:], scalar1=255.0,
                                    scalar2=None, op0=Alu.is_equal)
            s01 = sb.tile([P, P], F32)
            nc.vector.tensor_tensor(out=s01[:], in0=e0[:], in1=e255[:], op=Alu.add)
            fm = sb.tile([P, P], F32)  # fsss_mask
            nc.vector.tensor_scalar(out=fm[:], in0=s01[:], scalar1=-1.0,
                                    scalar2=1.0, op0=Alu.mult, op1=Alu.add)
            am = sb.tile([P, P], F32)  # anchor
            nc.vector.tensor_tensor(out=am[:], in0=pm[:], in1=fm[:], op=Alu.mult)
            pom = sb.tile([P, P], F32)  # positive
            nc.vector.tensor_tensor(out=pom[:], in0=fm[:], in1=am[:], op=Alu.subtract)
            nm = sb.tile([P, P], F32)  # negative
            nc.vector.tensor_tensor(out=nm[:], in0=pm[:], in1=e0[:], op=Alu.mult)
            masks = [am, pom, nm]

            # row sums + exclusive row prefix (strict lower-tri matmul)
            rs = sb.tile([P, 3], F32)
            for x, m in enumerate(masks):
                nc.vector.reduce_sum(out=rs[:, x:x + 1], in_=m[:], axis=AX.X)
            excl_ps = pst.tile([P, 3], F32, tag="smallps")
            nc.tensor.matmul(out=excl_ps[:], lhsT=lstrict[:], rhs=rs[:],
                             start=True, stop=True)
            excl_sb = sb.tile([P, 3], F32)
            nc.vector.tensor_copy(excl_sb[:], excl_ps[:])

            # within-core inclusive cumulative counts (batch-major row order)
            cums = []
            for x, m in enumerate(masks):
                cum = sb.tile([P, P], F32, name=f"cum{x}")
                nc.vector.tensor_tensor_scan(
                    out=cum[:], data0=m[:], data1=m[:],
                    initial=excl_sb[:, x:x + 1], op0=Alu.add, op1=Alu.bypass)
                cums.append(cum)

            # local totals -> AllGather -> per-core offsets / global totals
            tot_ps = pst.tile([1, 3], F32, tag="smallps")
            nc.tensor.matmul(out=tot_ps[:], lhsT=onescol[:], rhs=rs[:],
                             start=True, stop=True)
            cc_in = sb.tile([1, 32], F32)
            nc.vector.memset(cc_in[:], 0.0)
            nc.vector.tensor_copy(cc_in[:, 0:3], tot_ps[:])
            ag_in_d = dr.tile([1, 32], F32)
            ag_out_d = dr.tile([8, 32], F32, addr_space="Shared")
            nc.sync.dma_start(out=ag_in_d[:], in_=cc_in[:])
            real_ag = nc.gpsimd.collective_compute(
                "AllGather", Alu.bypass, replica_groups=[CORE_IDS],
                ins=[ag_in_d[:]], outs=[ag_out_d[:]])
            add_dep_helper(real_ag.ins, warm_cc.ins,
                           reason="warmup collective first")
            cnts8 = sb.tile([8, 32], F32)
            nc.sync.dma_start(out=cnts8[:], in_=ag_out_d[:])

            pid_sb = sb.tile([1, 1], U32)
            nc.sync.dma_start(out=pid_sb[:], in_=nc.partition_id_tensor[0:1, 0:1])
            pid_f = sb.tile([1, 1], F32)
            nc.vector.tensor_copy(pid_f[:], pid_sb[:])
            pidb_ps = pst.tile([8, 1], F32, tag="smallps")
            nc.tensor.matmul(out=pidb_ps[:], lhsT=ones8r[:], rhs=pid_f[:],
                             start=True, stop=True)
            ltm = sb.tile([8, 1], F32)
            nc.vector.tensor_tensor(out=ltm[:], in0=iota8[:], in1=pidb_ps[:],
                                    op=Alu.is_lt)
            offs_ps = pst.tile([1, 3], F32, tag="smallps")
            nc.tensor.matmul(out=offs_ps[:], lhsT=ltm[:], rhs=cnts8[:, 0:3],
                             start=True, stop=True)
            gtot_ps = pst.tile([1, 3], F32, tag="smallps")
            nc.tensor.matmul(out=gtot_ps[:], lhsT=ones8[:], rhs=cnts8[:, 0:3],
                             start=True, stop=True)
            gtot = sb.tile([1, 3], F32)
            nc.vector.tensor_copy(gtot[:], gtot_ps[:])
            # C3 = K - offset (selection cutoffs for this core)
            c3row = sb.tile([1, 3], F32)
            nc.vector.tensor_tensor(out=c3row[:], in0=k3row[:], in1=offs_ps[:],
                                    op=Alu.subtract)
            c3b_ps = pst.tile([P, 3], F32, tag="smallps")
            nc.tensor.matmul(out=c3b_ps[:], lhsT=onesrow[:], rhs=c3row[:],
                             start=True, stop=True)
            c3b = sb.tile([P, 3], F32)
            nc.vector.tensor_copy(c3b[:], c3b_ps[:])

            # selection masks -> transposed [w, h] with PE, into one [P, 3*P]
            selT = sb.tile([P, 3 * P], F32)
            for x, m in enumerate(masks):
                selx = sb.tile([P, P], F32, name=f"selx{x}")
                nc.vector.scalar_tensor_tensor(
                    out=selx[:], in0=cums[x][:], scalar=c3b[:, x:x + 1],
                    in1=m[:], op0=Alu.is_le, op1=Alu.mult)
                selt_ps = pst.tile([P, P], F32, name="selt_ps", tag="smallps")
                nc.tensor.transpose(out=selt_ps[:], in_=selx[:], identity=ident[:])
                nc.scalar.copy(selT[:, x * P:(x + 1) * P], selt_ps[:])

            # ============== embedding stream: S3 += WV_t^T @ embT_t ==============
            wv = sb.tile([P, 3 * P], BF16)  # [w, x*128 + h] weights
            wv_re = wv[:].rearrange("p (x h) -> p h x", x=3)
            s3_qs = [ps.tile([3, P], F32, name=f"s3q{q}") for q in range(4)]
            for c in range(NCHUNK):
                ech = stream.tile([P, TPC * D], BF16, name="ech")
                nc.sync.dma_start(out=ech[:], in_=emb_in[c])
                sq = sqp.tile([P, TPC * D], BF16, name="sq")
                nc.scalar.square(sq[:], ech[:])
                ss16 = sqp.tile([P, TPC], F32, name="ss16")
                nc.vector.reduce_sum(
                    out=ss16[:], in_=sq[:].rearrange("p (t d) -> p t d", t=TPC),
                    axis=AX.X)
                nrm = sqp.tile([P, TPC], F32, name="nrm")
                nc.scalar.sqrt(nrm[:], ss16[:])
                nrmc = sqp.tile([P, TPC], F32, name="nrmc")
                nc.vector.tensor_scalar(out=nrmc[:], in0=nrm[:], scalar1=EPS,
                                        scalar2=None, op0=Alu.max)
                inv16 = sqp.tile([P, TPC], F32, name="inv16")
                nc.vector.reciprocal(inv16[:], nrmc[:])
                wv3 = wv[:].rearrange("p (x h) -> p x h", x=3)
                selT3 = selT[:].rearrange("p (x h) -> p x h", x=3)
                inv_b = inv16[:].rearrange("p (o t) -> p o t", o=1).to_broadcast(
                    [P, 3, TPC])
                nc.vector.tensor_tensor(
                    out=wv3[:, :, c * TPC:(c + 1) * TPC],
                    in0=selT3[:, :, c * TPC:(c + 1) * TPC],
                    in1=inv_b, op=Alu.mult)
                for tt in range(TPC):
                    t = c * TPC + tt
                    q = t % 4
                    nc.tensor.matmul(
                        out=s3_qs[q][:], lhsT=wv_re[:, t, :],
                        rhs=ech[:, tt * D:(tt + 1) * D],
                        start=(t < 4), stop=(t >= P - 4))

            # ============== AllReduce S3 ==============
            s3_sb = sb.tile([3, P], F32)
            nc.scalar.copy(s3_sb[:], s3_qs[0][:])
            for q in range(1, 4):
                nc.vector.tensor_tensor(out=s3_sb[:], in0=s3_sb[:],
                                        in1=s3_qs[q][:], op=Alu.add)
            ar_in_d = dr.tile([3, P], F32)
            ar_out_d = dr.tile([3, P], F32, addr_space="Shared")
            nc.sync.dma_start(out=ar_in_d[:], in_=s3_sb[:])
            nc.gpsimd.collective_compute(
                "AllReduce", Alu.add, replica_groups=[CORE_IDS],
                ins=[ar_in_d[:]], outs=[ar_out_d[:]])
            s3g = sb.tile([1, 3 * P], F32)  # rows flattened onto partition 0
            nc.sync.dma_start(
                out=s3g[:].rearrange("o (r d) -> (o r) d", r=3),
                in_=ar_out_d[:])

            # ============== memory-bank base sums ==============
            # n3 = min(total, K); rows >= n3 keep their original value
            n3 = sb.tile([1, 3], F32)
            nc.vector.tensor_tensor(out=n3[:], in0=gtot[:], in1=k3row[:], op=Alu.min)
            n3b_ps = pst.tile([P, 3], F32, tag="smallps")
            nc.tensor.matmul(out=n3b_ps[:], lhsT=onesrow[:], rhs=n3[:],
                             start=True, stop=True)
            n3b = sb.tile([P, 3], F32)
            nc.vector.tensor_copy(n3b[:], n3b_ps[:])

            pmem = sb.tile([P, 8 * D], BF16)
            nc.sync.dma_start(out=pmem[:], in_=pmem_in[:])
            nmem = sb.tile([P, 8 * D], BF16)
            nc.sync.dma_start(out=nmem[:], in_=nmem_in[:])
            rm_pos = sb.tile([P, 8], BF16)
            nc.vector.tensor_scalar(out=rm_pos[:], in0=iotam[:],
                                    scalar1=n3b[:, 1:2], scalar2=None,
                                    op0=Alu.is_ge)
            rm_neg = sb.tile([P, 8], BF16)
            nc.vector.tensor_scalar(out=rm_neg[:], in0=iotam[:],
                                    scalar1=n3b[:, 2:3], scalar2=None,
                                    op0=Alu.is_ge)
            bpos_ps = ps.tile([1, D], F32)
            bneg_ps = ps.tile([1, D], F32)
            for t in range(8):
                nc.tensor.matmul(out=bpos_ps[:], lhsT=rm_pos[:, t:t + 1],
                                 rhs=pmem[:, t * D:(t + 1) * D],
                                 start=(t == 0), stop=(t == 7))
            for t in range(8):
                nc.tensor.matmul(out=bneg_ps[:], lhsT=rm_neg[:, t:t + 1],
                                 rhs=nmem[:, t * D:(t + 1) * D],
                                 start=(t == 0), stop=(t == 7))

            # ============== final scalar ==============
            prow = sb.tile([1, D], F32)
            nc.vector.tensor_tensor(out=prow[:], in0=s3g[:, P:2 * P],
                                    in1=bpos_ps[:], op=Alu.add)
            nrow = sb.tile([1, D], F32)
            nc.vector.tensor_tensor(out=nrow[:], in0=s3g[:, 2 * P:3 * P],
                                    in1=bneg_ps[:], op=Alu.add)
            mp = sb.tile([1, D], F32)
            nc.vector.tensor_tensor(out=mp[:], in0=s3g[:, 0:P], in1=prow[:],
                                    op=Alu.mult)
            dotp = sb.tile([1, 1], F32)
            nc.vector.reduce_sum(out=dotp[:], in_=mp[:], axis=AX.X)
            mn = sb.tile([1, D], F32)
            nc.vector.tensor_tensor(out=mn[:], in0=s3g[:, 0:P], in1=nrow[:],
                                    op=Alu.mult)
            dotn = sb.tile([1, 1], F32)
            nc.vector.reduce_sum(out=dotn[:], in_=mn[:], axis=AX.X)
            diff = sb.tile([1, 1], F32)
            nc.vector.tensor_tensor(out=diff[:], in0=dotp[:], in1=dotn[:],
                                    op=Alu.subtract)
            nanc = sb.tile([1, 1], F32)
            nc.vector.tensor_scalar(out=nanc[:], in0=n3[:, 0:1], scalar1=1.0,
                                    scalar2=float(MEM), op0=Alu.max, op1=Alu.mult)
            rec = sb.tile([1, 1], F32)
            nc.vector.reciprocal(rec[:], nanc[:])
            res = sb.tile([1, 1], F32)
            nc.vector.tensor_tensor(out=res[:], in0=diff[:], in1=rec[:],
                                    op=Alu.mult)
            resf = sb.tile([1, 1], F32)
            nc.vector.tensor_scalar(out=resf[:], in0=res[:], scalar1=MARGIN,
                                    scalar2=0.0, op0=Alu.add, op1=Alu.max)
            nc.sync.dma_start(out=out_t[:], in_=resf[:])

    nc.compile()
    return nc


def _consts():
    cst = np.zeros((P, 400), np.float32)
    cst[:, 0:128] = np.eye(P, dtype=np.float32)
    cst[:, 128:256] = np.triu(np.ones((P, P), np.float32), 1)
    cst[:, 256:264] = (np.arange(8)[None, :] * P
                       + np.arange(P)[:, None]).astype(np.float32)
    cst[:, 264] = 1.0                       # ones column (onescol / ones8)
    cst[0, 265:393] = 1.0                   # ones row (onesrow / ones8r)
    cst[0:8, 393] = np.arange(8)            # iota8 column
    cst[0, 394:397] = [K_ANC, K_POS, K_NEG]
    return dict(c_all=cst)


def _shard(preds, embeddings, fsss_gts, pos_memory, neg_memory):
    consts = _consts()
    pmem_pad = np.zeros((1024, D), np.float32)
    pmem_pad[:MEM] = pos_memory
    nmem_pad = np.zeros((1024, D), np.float32)
    nmem_pad[:MEM] = neg_memory
    pmem_h = np.ascontiguousarray(
        pmem_pad.reshape(8, P, D).transpose(1, 0, 2)).reshape(P, 8 * D)
    pmem_h = pmem_h.astype(ml_dtypes.bfloat16)
    nmem_h = np.ascontiguousarray(
        nmem_pad.reshape(8, P, D).transpose(1, 0, 2)).reshape(P, 8 * D)
    nmem_h = nmem_h.astype(ml_dtypes.bfloat16)

    in_maps = []
    for b in range(N_CORES):
        pr = np.ascontiguousarray(
            preds[b][:, ::4, ::4].transpose(1, 0, 2)).reshape(P, NCH * P)
        fs = np.ascontiguousarray(fsss_gts[b][::4, ::4]).astype(np.int32)
        # emb chunk layout: [c, w, t*128 + ch] = emb[ch, c*16 + t, w]
        eh = np.ascontiguousarray(
            embeddings[b].reshape(D, NCHUNK, TPC, P).transpose(1, 3, 2, 0)
        ).reshape(NCHUNK, P, TPC * D).astype(ml_dtypes.bfloat16)
        m = dict(p_preds=pr.astype(np.float32, copy=False),
                 p_fsss=fs, p_emb=eh,
                 p_pmem=pmem_h, p_nmem=nmem_h)
        m.update(consts)
        in_maps.append(m)
    return in_maps


_NC_CACHE = None


def _get_nc():
    global _NC_CACHE
    if _NC_CACHE is None:
        _NC_CACHE = _build()
    return _NC_CACHE


def kernel(preds, embeddings, gts, fsss_gts, pos_memory, neg_memory, **_ignored):
    preds = np.asarray(preds, dtype=np.float32)
    embeddings = np.asarray(embeddings, dtype=np.float32)
    fsss_gts = np.asarray(fsss_gts)
    pos_memory = np.asarray(pos_memory, dtype=np.float32)
    neg_memory = np.asarray(neg_memory, dtype=np.float32)
    in_maps = _shard(preds, embeddings, fsss_gts, pos_memory, neg_memory)
    res = run_bass_kernel_spmd(_get_nc(), in_maps, CORE_IDS)
    return np.float32(res.results[0]["out"][0, 0])


def run_traced(tmpdir=None, **inputs):
    """test.py helper: run with NTFF tracing, return (value, BassKernelResults)."""
    in_maps = _shard(
        np.asarray(inputs["preds"], np.float32),
        np.asarray(inputs["embeddings"], np.float32),
        np.asarray(inputs["fsss_gts"]),
        np.asarray(inputs["pos_memory"], np.float32),
        np.asarray(inputs["neg_memory"], np.float32),
    )
    res = run_bass_kernel_spmd(_get_nc(), in_maps, CORE_IDS, trace=True,
                               trace_cores=CORE_IDS, stitch_traces=True,
                               tmpdir=tmpdir)
    return np.float32(res.results[0]["out"][0, 0]), res



# revision 14
# speedup vs baseline: 1.3562x; 1.3562x over previous
"""BoundaryLoss TRN2 kernel — 8-core SPMD Bass/Tile implementation.

Strategy (data-parallel over batch B=8, one batch per NeuronCore):
  * Only every 4th pixel of preds/fsss_gts matters (nearest 512->128 downsample
    = stride-4 selection), so the host shards preds[:, :, ::4, ::4] etc.
  * The scatter into the circular memory banks only matters through column
    sums: sum(pos_mem) = sum(pos_memory[n_pos:]) + sum(selected E rows),
    n_pos = min(#selected, 333).  No scatter is materialised.
  * first-k selection over the global batch-major flattening = per-core
    cumsum + exclusive prefix of per-core counts (one tiny AllGather).
  * Per core: weighted sums S_x = sum_n sel_x[n] * E[n, :] computed as 128
    PE matmuls over transposed embedding tiles with per-position weights
    sel_x * 1/max(||emb_n||, eps); one AllReduce of the [3,128] partials;
    the final scalar is computed redundantly on every core.

Self-contained: hardcodes all shapes for the nn_BoundaryLoss problem
(B=8, C=21, H=W=512, D=128, h=w=128, MEM=1000).
"""

import ml_dtypes
import numpy as np

import concourse.bass as bass
import concourse.bacc as bacc
import concourse.mybir as mybir
import concourse.tile as tile
from concourse.bass_utils import run_bass_kernel_spmd

F32 = mybir.dt.float32
BF16 = mybir.dt.bfloat16
I32 = mybir.dt.int32
U32 = mybir.dt.uint32
Alu = mybir.AluOpType
AX = mybir.AxisListType

N_CORES = 8
CORE_IDS = list(range(N_CORES))
P = 128          # partitions / feature spatial dim
NCH = 21         # pred classes
D = 128          # embedding dim
MEM = 1000
K_ANC, K_POS, K_NEG = MEM // 10, MEM // 3, MEM // 3
MARGIN = 0.2
EPS = 1e-12
NCHUNK = 8       # embedding chunks per core (16 tiles of 128 positions each)
TPC = 16         # tiles per chunk


def _build():
    nc = bacc.Bacc("TRN2", target_bir_lowering=False, debug=False,
                   num_devices=N_CORES)

    # ---- per-core sharded inputs ----
    preds_in = nc.declare_dram_parameter("p_preds", [P, NCH * P], F32, isOutput=False)
    fsss_in = nc.declare_dram_parameter("p_fsss", [P, P], I32, isOutput=False)
    emb_in = nc.declare_dram_parameter("p_emb", [NCHUNK, P, TPC * D], BF16, isOutput=False)
    pmem_in = nc.declare_dram_parameter("p_pmem", [P, 8 * D], BF16, isOutput=False)
    nmem_in = nc.declare_dram_parameter("p_nmem", [P, 8 * D], BF16, isOutput=False)
    ltm_in = nc.declare_dram_parameter("p_ltm", [8, 1], F32, isOutput=False)
    # ---- replicated constants (one packed tensor) ----
    cst_in = nc.declare_dram_parameter("c_all", [P, 400], F32, isOutput=False)

    out_t = nc.declare_dram_parameter("out", [1, 1], F32, isOutput=True)

    with tile.TileContext(nc) as tc:
        with (
            tc.tile_pool(name="sb", bufs=1) as sb,
            tc.tile_pool(name="stream", bufs=8) as stream,
            tc.tile_pool(name="sq", bufs=2) as sqp,
            tc.tile_pool(name="ps", bufs=1, space="PSUM") as ps,
            tc.tile_pool(name="pst", bufs=2, space="PSUM") as pst,
            tc.tile_pool(name="dr", bufs=1, space="DRAM") as dr,
        ):
            # preds/fsss first in the DMA queue (mask phase is the critical path)
            preds_sb = sb.tile([P, NCH * P], F32)
            nc.sync.dma_start(out=preds_sb[:], in_=preds_in[:])
            fsss_sb = sb.tile([P, P], I32)
            nc.sync.dma_start(out=fsss_sb[:], in_=fsss_in[:])

            # ============== constants (single DMA) ==============
            cst = sb.tile([P, 400], F32)
            nc.sync.dma_start(out=cst[:], in_=cst_in[:])
            ltm = sb.tile([8, 1], F32)
            nc.sync.dma_start(out=ltm[:], in_=ltm_in[:])
            ident = cst[:, 0:128]
            lstrict = cst[:, 128:256]
            iotam = cst[:, 256:264]
            onescol = cst[:, 264:265]
            ones8 = cst[0:8, 264:265]
            onesrow = cst[0:1, 265:393]
            k3row = cst[0:1, 394:397]
            apat = cst[0:24, 397:398]
            wpat = cst[0:24, 398:399]

            # ============== mask phase (grids are [h, w]) ==============
            preds_re = preds_sb[:].rearrange("p (c w) -> p w c", c=NCH)
            mx = sb.tile([P, P], F32)
            nc.vector.reduce_max(out=mx[:], in_=preds_re[:, :, 1:NCH], axis=AX.X)
            pm = sb.tile([P, P], F32)  # pred_mask
            nc.vector.tensor_tensor(out=pm[:], in0=mx[:], in1=preds_re[:, :, 0],
                                    op=Alu.is_gt)

            fsf = sb.tile([P, P], F32)
            nc.vector.tensor_copy(fsf[:], fsss_sb[:])
            e0 = sb.tile([P, P], F32)
            nc.vector.tensor_scalar(out=e0[:], in0=fsf[:], scalar1=0.0,
                                    scalar2=None, op0=Alu.is_equal)
            e255 = sb.tile([P, P], F32)
            nc.vector.tensor_scalar(out=e255[:], in0=fsf[:], scalar1=255.0,
                                    scalar2=None, op0=Alu.is_equal)
            s01 = sb.tile([P, P], F32)
            nc.vector.tensor_tensor(out=s01[:], in0=e0[:], in1=e255[:], op=Alu.add)
            fm = sb.tile([P, P], F32)  # fsss_mask
            nc.vector.tensor_scalar(out=fm[:], in0=s01[:], scalar1=-1.0,
                                    scalar2=1.0, op0=Alu.mult, op1=Alu.add)
            am = sb.tile([P, P], F32)  # anchor
            nc.vector.tensor_tensor(out=am[:], in0=pm[:], in1=fm[:], op=Alu.mult)
            pom = sb.tile([P, P], F32)  # positive
            nc.vector.tensor_tensor(out=pom[:], in0=fm[:], in1=am[:], op=Alu.subtract)
            nm = sb.tile([P, P], F32)  # negative
            nc.vector.tensor_tensor(out=nm[:], in0=pm[:], in1=e0[:], op=Alu.mult)
            masks = [am, pom, nm]

            # row sums + exclusive row prefix (strict lower-tri matmul)
            rs = sb.tile([P, 3], F32)
            for x, m in enumerate(masks):
                nc.vector.reduce_sum(out=rs[:, x:x + 1], in_=m[:], axis=AX.X)
            excl_ps = pst.tile([P, 3], F32, tag="smallps")
            nc.tensor.matmul(out=excl_ps[:], lhsT=lstrict[:], rhs=rs[:],
                             start=True, stop=True)
            excl_sb = sb.tile([P, 3], F32)
            nc.vector.tensor_copy(excl_sb[:], excl_ps[:])

            # within-core inclusive cumulative counts (batch-major row order)
            cums = []
            for x, m in enumerate(masks):
                cum = sb.tile([P, P], F32, name=f"cum{x}")
                nc.vector.tensor_tensor_scan(
                    out=cum[:], data0=m[:], data1=m[:],
                    initial=excl_sb[:, x:x + 1], op0=Alu.add, op1=Alu.bypass)
                cums.append(cum)

            # local totals -> AllGather (the first collective: its trigger at
            # ~17us local still beats the ~18us CC-core boot, so no warmup
            # collective is needed to absorb the cold-init)
            tot_ps = pst.tile([1, 3], F32, tag="smallps")
            nc.tensor.matmul(out=tot_ps[:], lhsT=onescol[:], rhs=rs[:],
                             start=True, stop=True)
            cc_in = sb.tile([1, 32], F32)
            nc.vector.memset(cc_in[:], 0.0)
            nc.vector.tensor_copy(cc_in[:, 0:3], tot_ps[:])
            ag_in_d = dr.tile([1, 32], F32)
            ag_out_d = dr.tile([8, 32], F32, addr_space="Shared")
            nc.sync.dma_start(out=ag_in_d[:], in_=cc_in[:])
            nc.gpsimd.collective_compute(
                "AllGather", Alu.bypass, replica_groups=[CORE_IDS],
                ins=[ag_in_d[:]], outs=[ag_out_d[:]])
            cnts8 = sb.tile([8, 32], F32)
            nc.sync.dma_start(out=cnts8[:], in_=ag_out_d[:])

            # Pre-AG: fold each mask into its cumsum (non-mask rows -> +BIG so
            # any cutoff excludes them) and transpose to [w, h] layout with PE.
            cumMT = sb.tile([P, 3 * P], F32)
            for x, m in enumerate(masks):
                big = sb.tile([P, P], F32, name=f"big{x}")
                nc.vector.tensor_scalar(out=big[:], in0=m[:], scalar1=0.0,
                                        scalar2=1e9, op0=Alu.is_equal,
                                        op1=Alu.mult)
                cumm = sb.tile([P, P], F32, name=f"cumm{x}")
                nc.vector.tensor_tensor(out=cumm[:], in0=cums[x][:], in1=big[:],
                                        op=Alu.add)
                cumt_ps = pst.tile([P, P], F32, name="cumt_ps", tag="smallps")
                nc.tensor.transpose(out=cumt_ps[:], in_=cumm[:], identity=ident[:])
                nc.scalar.copy(cumMT[:, x * P:(x + 1) * P], cumt_ps[:])

            # ============== embedding stream: norms only (pre-AG) ==============
            invT = sb.tile([P, P], F32)  # [w, h] 1/max(||emb||, eps)
            echs = []
            for c in range(NCHUNK):
                ech = stream.tile([P, TPC * D], BF16, name="ech")
                nc.sync.dma_start(out=ech[:], in_=emb_in[c])
                echs.append(ech)
                sq = sqp.tile([P, TPC * D], BF16, name="sq")
                nc.scalar.square(sq[:], ech[:])
                ss16 = sqp.tile([P, TPC], F32, name="ss16")
                nc.vector.reduce_sum(
                    out=ss16[:], in_=sq[:].rearrange("p (t d) -> p t d", t=TPC),
                    axis=AX.X)
                nrm = sqp.tile([P, TPC], F32, name="nrm")
                nc.scalar.sqrt(nrm[:], ss16[:])
                nrmc = sqp.tile([P, TPC], F32, name="nrmc")
                nc.vector.tensor_scalar(out=nrmc[:], in0=nrm[:], scalar1=EPS,
                                        scalar2=None, op0=Alu.max)
                nc.vector.reciprocal(invT[:, c * TPC:(c + 1) * TPC], nrmc[:])

            pmem = sb.tile([P, 8 * D], BF16)
            nc.sync.dma_start(out=pmem[:], in_=pmem_in[:])
            nmem = sb.tile([P, 8 * D], BF16)
            nc.sync.dma_start(out=nmem[:], in_=nmem_in[:])

            # ============== post-AG: cutoffs -> weights -> S3 matmuls ==========
            offs_ps = pst.tile([1, 3], F32, tag="smallps")
            nc.tensor.matmul(out=offs_ps[:], lhsT=ltm[:], rhs=cnts8[:, 0:3],
                             start=True, stop=True)
            gtot_ps = pst.tile([1, 3], F32, tag="smallps")
            nc.tensor.matmul(out=gtot_ps[:], lhsT=ones8[:], rhs=cnts8[:, 0:3],
                             start=True, stop=True)
            # C3 = K - offset (selection cutoffs for this core)
            c3row = sb.tile([1, 3], F32)
            nc.vector.tensor_tensor(out=c3row[:], in0=k3row[:], in1=offs_ps[:],
                                    op=Alu.subtract)
            c3b_ps = pst.tile([P, 3], F32, tag="smallps")
            nc.tensor.matmul(out=c3b_ps[:], lhsT=onesrow[:], rhs=c3row[:],
                             start=True, stop=True)
            c3b = sb.tile([P, 3], F32)
            nc.vector.tensor_copy(c3b[:], c3b_ps[:])

            wv = sb.tile([P, 3 * P], BF16)  # [w, x*128 + h] weights
            wv_re = wv[:].rearrange("p (x h) -> p h x", x=3)
            wv3 = wv[:].rearrange("p (x h) -> p x h", x=3)
            cumMT3 = cumMT[:].rearrange("p (x h) -> p x h", x=3)
            inv_b = invT[:].rearrange("p (o h) -> p o h", o=1).to_broadcast(
                [P, 3, P])
            for x in range(3):
                nc.vector.scalar_tensor_tensor(
                    out=wv3[:, x, :], in0=cumMT3[:, x, :],
                    scalar=c3b[:, x:x + 1], in1=inv_b[:, x, :],
                    op0=Alu.is_le, op1=Alu.mult)

            s3_qs = [ps.tile([3, P], F32, name=f"s3q{q}") for q in range(4)]
            for c in range(NCHUNK):
                for tt in range(TPC):
                    t = c * TPC + tt
                    q = t % 4
                    nc.tensor.matmul(
                        out=s3_qs[q][:], lhsT=wv_re[:, t, :],
                        rhs=echs[c][:, tt * D:(tt + 1) * D],
                        start=(t < 4), stop=(t >= P - 4))

            # ============== memory-bank base sums (scaled 1/8) ==============
            # n3 = min(total, K); rows >= n3 keep their original value.  Every
            # core adds base/8 to its partial so the AllGather-sum yields the
            # full base exactly once.
            n3 = sb.tile([1, 3], F32)
            nc.vector.tensor_tensor(out=n3[:], in0=gtot_ps[:], in1=k3row[:],
                                    op=Alu.min)
            n3b_ps = pst.tile([P, 3], F32, tag="smallps")
            nc.tensor.matmul(out=n3b_ps[:], lhsT=onesrow[:], rhs=n3[:],
                             start=True, stop=True)
            n3b = sb.tile([P, 3], F32)
            nc.vector.tensor_copy(n3b[:], n3b_ps[:])

            rmp = sb.tile([P, 24], BF16)
            nc.vector.memset(rmp[:], 0.0)
            rmn = sb.tile([P, 24], BF16)
            nc.vector.memset(rmn[:], 0.0)
            rmp_re = rmp[:].rearrange("p (t c) -> p t c", c=3)
            rmn_re = rmn[:].rearrange("p (t c) -> p t c", c=3)
            nc.vector.tensor_scalar(out=rmp_re[:, :, 1], in0=iotam[:],
                                    scalar1=n3b[:, 1:2], scalar2=0.125,
                                    op0=Alu.is_ge, op1=Alu.mult)
            nc.vector.tensor_scalar(out=rmn_re[:, :, 2], in0=iotam[:],
                                    scalar1=n3b[:, 2:3], scalar2=0.125,
                                    op0=Alu.is_ge, op1=Alu.mult)
            bps = ps.tile([3, P], F32, name="bps")
            for t in range(8):
                nc.tensor.matmul(out=bps[:], lhsT=rmp[:, 3 * t:3 * t + 3],
                                 rhs=pmem[:, t * D:(t + 1) * D],
                                 start=(t == 0), stop=False)
            for t in range(8):
                nc.tensor.matmul(out=bps[:], lhsT=rmn[:, 3 * t:3 * t + 3],
                                 rhs=nmem[:, t * D:(t + 1) * D],
                                 start=False, stop=(t == 7))

            # ============== AllGather partials (anc, pos+base/8, neg+base/8) ==
            s01a = sb.tile([3, P], F32, name="s3a")
            nc.scalar.copy(s01a[:], s3_qs[0][:])
            nc.vector.tensor_tensor(out=s01a[:], in0=s01a[:],
                                    in1=s3_qs[1][:], op=Alu.add)
            s23a = sb.tile([3, P], F32, name="s3b")
            nc.scalar.copy(s23a[:], s3_qs[2][:])
            nc.vector.tensor_tensor(out=s23a[:], in0=s23a[:],
                                    in1=s3_qs[3][:], op=Alu.add)
            sq3 = sb.tile([3, P], F32, name="s3c")
            nc.vector.tensor_tensor(out=sq3[:], in0=s01a[:], in1=s23a[:],
                                    op=Alu.add)
            s3_sb = sb.tile([3, P], F32)
            nc.vector.tensor_tensor(out=s3_sb[:], in0=sq3[:], in1=bps[:],
                                    op=Alu.add)
            ar_in_d = dr.tile([3, P], F32)
            ar_out_d = dr.tile([24, P], F32, addr_space="Shared")
            nc.sync.dma_start(out=ar_in_d[:], in_=s3_sb[:])
            nc.gpsimd.collective_compute(
                "AllGather", Alu.bypass, replica_groups=[CORE_IDS],
                ins=[ar_in_d[:]], outs=[ar_out_d[:]])
            g24 = sb.tile([24, P], F32)
            nc.sync.dma_start(out=g24[:], in_=ar_out_d[:])

            # ============== final scalar ==============
            # anc = apat^T g24, posneg = wpat^T g24; loss pre-scale = anc.posneg
            anc_ps = pst.tile([1, P], F32, tag="smallps")
            nc.tensor.matmul(out=anc_ps[:], lhsT=apat[:], rhs=g24[:],
                             start=True, stop=True)
            pn_ps = pst.tile([1, P], F32, tag="smallps")
            nc.tensor.matmul(out=pn_ps[:], lhsT=wpat[:], rhs=g24[:],
                             start=True, stop=True)
            anc_sb = sb.tile([1, D], F32)
            nc.scalar.copy(anc_sb[:], anc_ps[:])
            mp = sb.tile([1, D], F32)
            nc.vector.tensor_tensor(out=mp[:], in0=anc_sb[:], in1=pn_ps[:],
                                    op=Alu.mult)
            diff = sb.tile([1, 1], F32)
            nc.vector.reduce_sum(out=diff[:], in_=mp[:], axis=AX.X)
            nanc = sb.tile([1, 1], F32)
            nc.vector.tensor_scalar(out=nanc[:], in0=n3[:, 0:1], scalar1=1.0,
                                    scalar2=float(MEM), op0=Alu.max, op1=Alu.mult)
            rec = sb.tile([1, 1], F32)
            nc.vector.reciprocal(rec[:], nanc[:])
            res = sb.tile([1, 1], F32)
            nc.vector.tensor_tensor(out=res[:], in0=diff[:], in1=rec[:],
                                    op=Alu.mult)
            resf = sb.tile([1, 1], F32)
            nc.vector.tensor_scalar(out=resf[:], in0=res[:], scalar1=MARGIN,
                                    scalar2=0.0, op0=Alu.add, op1=Alu.max)
            nc.sync.dma_start(out=out_t[:], in_=resf[:])

    nc.compile()
    return nc


def _consts():
    cst = np.zeros((P, 400), np.float32)
    cst[:, 0:128] = np.eye(P, dtype=np.float32)
    cst[:, 128:256] = np.triu(np.ones((P, P), np.float32), 1)
    cst[:, 256:264] = (np.arange(8)[None, :] * P
                       + np.arange(P)[:, None]).astype(np.float32)
    cst[:, 264] = 1.0                       # ones column (onescol / ones8)
    cst[0, 265:393] = 1.0                   # ones row (onesrow)
    cst[0, 394:397] = [K_ANC, K_POS, K_NEG]
    j = np.arange(24)
    cst[0:24, 397] = (j % 3 == 0)           # apat: pick anchor rows
    cst[0:24, 398] = (j % 3 == 1).astype(np.float32) - (j % 3 == 2)  # wpat
    return dict(c_all=cst)


def _shard(preds, embeddings, fsss_gts, pos_memory, neg_memory):
    consts = _consts()
    pmem_pad = np.zeros((1024, D), np.float32)
    pmem_pad[:MEM] = pos_memory
    nmem_pad = np.zeros((1024, D), np.float32)
    nmem_pad[:MEM] = neg_memory
    pmem_h = np.ascontiguousarray(
        pmem_pad.reshape(8, P, D).transpose(1, 0, 2)).reshape(P, 8 * D)
    pmem_h = pmem_h.astype(ml_dtypes.bfloat16)
    nmem_h = np.ascontiguousarray(
        nmem_pad.reshape(8, P, D).transpose(1, 0, 2)).reshape(P, 8 * D)
    nmem_h = nmem_h.astype(ml_dtypes.bfloat16)

    in_maps = []
    for b in range(N_CORES):
        pr = np.ascontiguousarray(
            preds[b][:, ::4, ::4].transpose(1, 0, 2)).reshape(P, NCH * P)
        fs = np.ascontiguousarray(fsss_gts[b][::4, ::4]).astype(np.int32)
        # emb chunk layout: [c, w, t*128 + ch] = emb[ch, c*16 + t, w]
        eh = np.ascontiguousarray(
            embeddings[b].reshape(D, NCHUNK, TPC, P).transpose(1, 3, 2, 0)
        ).reshape(NCHUNK, P, TPC * D).astype(ml_dtypes.bfloat16)
        ltm = (np.arange(8) < b).astype(np.float32).reshape(8, 1)
        m = dict(p_preds=pr.astype(np.float32, copy=False),
                 p_fsss=fs, p_emb=eh,
                 p_pmem=pmem_h, p_nmem=nmem_h, p_ltm=ltm)
        m.update(consts)
        in_maps.append(m)
    return in_maps


_NC_CACHE = None


def _get_nc():
    global _NC_CACHE
    if _NC_CACHE is None:
        _NC_CACHE = _build()
    return _NC_CACHE


def kernel(preds, embeddings, gts, fsss_gts, pos_memory, neg_memory, **_ignored):
    preds = np.asarray(preds, dtype=np.float32)
    embeddings = np.asarray(embeddings, dtype=np.float32)
    fsss_gts = np.asarray(fsss_gts)
    pos_memory = np.asarray(pos_memory, dtype=np.float32)
    neg_memory = np.asarray(neg_memory, dtype=np.float32)
    in_maps = _shard(preds, embeddings, fsss_gts, pos_memory, neg_memory)
    res = run_bass_kernel_spmd(_get_nc(), in_maps, CORE_IDS)
    return np.float32(res.results[0]["out"][0, 0])


def run_traced(tmpdir=None, **inputs):
    """test.py helper: run with NTFF tracing, return (value, BassKernelResults)."""
    in_maps = _shard(
        np.asarray(inputs["preds"], np.float32),
        np.asarray(inputs["embeddings"], np.float32),
        np.asarray(inputs["fsss_gts"]),
        np.asarray(inputs["pos_memory"], np.float32),
        np.asarray(inputs["neg_memory"], np.float32),
    )
    res = run_bass_kernel_spmd(_get_nc(), in_maps, CORE_IDS, trace=True,
                               trace_cores=CORE_IDS, stitch_traces=True,
                               tmpdir=tmpdir)
    return np.float32(res.results[0]["out"][0, 0]), res

